# revision 1
# baseline (speedup 1.0000x reference)
"""CDWCE loss kernel for Trainium2 (8 NeuronCores, data-parallel over batch).

Math: loss = mean_b sum_j -log(1 - softmax(x)_bj + eps) * |j - t_b|^6
With u_bj = s_b - e_bj (s = row sum of exp), the per-element term is
v_bj = ln(s_b) - ln(u_bj)  (= -ln(1 - p_bj); the 1e-7 eps shifts the
reference value by <= ~1e-7 relative, far below tolerance).

dist |j-t|^6 only depends on the target class, so the dist-weighted sum is
applied AFTER aggregating per-class sums of ln(u) rows:
  loss*B = sum_c [ R_c * sum_{b:t=c} ln s_b ] - sum_{c,j} DIST[c,j] * sum_{b:t=c} ln u_bj
The per-class sums are computed on-chip with one-hot matmuls accumulating in
PSUM; the tiny 32x32 DIST contraction happens on the host in float64.

Engine split per tile: ACT exp+ln, DVE grouped reduce + one-hot compare,
GPSIMD broadcast-subtract (runs concurrently with DVE), PE scatter matmuls.
"""

import numpy as np

B, C = 1048576, 32
N_CORES = 8
B_LOCAL = B // N_CORES          # 131072 rows per core
P = 128                         # SBUF partitions
G = 128                         # rows per partition per tile
NT = B_LOCAL // (P * G)         # 8 tiles per core
F = G * C                       # 4096 f32 per partition per tile
QG = G // 4                     # 32 matmul groups per tile (4 rows each)
NCOL = 132                      # 4*32 ln(u) cols + 4 ln(s) cols per group
ALPHA = 6
Q_POOL = 32                     # how many of the QG u-subtract groups go to GPSIMD

_PROG = None


def _build_program():
    import concourse.bass as bass
    import concourse.bacc as bacc
    import concourse.tile as tile
    from concourse import mybir

    AF = mybir.ActivationFunctionType
    Alu = mybir.AluOpType
    f32 = mybir.dt.float32

    nc = bacc.Bacc("TRN2", target_bir_lowering=False, debug=False,
                   enable_asserts=True)
    x = nc.dram_tensor("x", [B_LOCAL, C], f32, kind="ExternalInput").ap()
    t = nc.dram_tensor("t", [B_LOCAL], f32, kind="ExternalInput").ap()
    iota = nc.dram_tensor("iota", [P, C], f32, kind="ExternalInput").ap()
    out = nc.dram_tensor("out", [P, NCOL], f32, kind="ExternalOutput").ap()

    # row index = n*(P*G) + p*G + g  ->  tile n, partition p, row-slot g
    xr = x.rearrange("(n p g) c -> n p (g c)", p=P, g=G)   # [NT, 128, F]
    tr = t.rearrange("(n p g) -> p n g", p=P, g=G)         # [128, NT, G]

    with tile.TileContext(nc) as tc:
        with (
            tc.tile_pool(name="consts", bufs=1) as consts,
            tc.tile_pool(name="xin", bufs=3) as xin,
            tc.tile_pool(name="work", bufs=2) as work,
            tc.tile_pool(name="smalls", bufs=2) as smalls,
            tc.tile_pool(name="psump", bufs=1, space="PSUM") as psump,
            tc.tile_pool(name="outp", bufs=1) as outp,
        ):
            iota_sb = consts.tile([P, C], f32)
            nc.sync.dma_start(out=iota_sb, in_=iota)
            t_sb = consts.tile([P, NT, G], f32)
            nc.sync.dma_start(out=t_sb, in_=tr)

            psum = psump.tile([P, NCOL], mybir.dt.float32)

            for i in range(NT):
                xt = xin.tile([P, F], f32)
                nc.sync.dma_start(out=xt, in_=xr[i])

                et = work.tile([P, F], f32)
                nc.scalar.activation(out=et, in_=xt, func=AF.Exp)

                st = smalls.tile([P, G], f32)
                nc.vector.reduce_sum(
                    out=st,
                    in_=et.rearrange("p (g c) -> p g c", c=C),
                    axis=mybir.AxisListType.X,
                )

                # ul[:, q, 0:128] = u for the 4 row-slots of group q
                # ul[:, q, 128:132] = ln(s) for those row-slots (filled below)
                ul = work.tile([P, QG, NCOL], f32)
                s4 = st.rearrange("p (q h) -> p q h", h=4)          # [P,QG,4]
                s4b = s4.unsqueeze(3).to_broadcast([P, QG, 4, C])
                e4 = et.rearrange("p (q h c) -> p q h c", h=4, c=C)
                u4 = ul[:, :, 0:128].rearrange("p q (h c) -> p q h c", c=C)
                if Q_POOL > 0:
                    nc.gpsimd.tensor_tensor(
                        out=u4[:, 0:Q_POOL], in0=s4b[:, 0:Q_POOL],
                        in1=e4[:, 0:Q_POOL], op=Alu.subtract)
                if Q_POOL < QG:
                    nc.vector.tensor_tensor(
                        out=u4[:, Q_POOL:], in0=s4b[:, Q_POOL:],
                        in1=e4[:, Q_POOL:], op=Alu.subtract)

                # ln(u) in place, ln(s) into the tail columns
                nc.scalar.activation(out=ul[:, :, 0:128], in_=ul[:, :, 0:128],
                                     func=AF.Ln)
                nc.scalar.activation(out=ul[:, :, 128:132], in_=s4, func=AF.Ln)

                # one-hot: oh[p, g*32+c] = (t[p,g] == c)
                oh = work.tile([P, F], f32)
                ib = iota_sb.unsqueeze(1).to_broadcast([P, G, C])
                tb = t_sb[:, i, :].unsqueeze(2).to_broadcast([P, G, C])
                nc.vector.tensor_tensor(
                    out=oh.rearrange("p (g c) -> p g c", c=C),
                    in0=ib, in1=tb, op=Alu.is_equal)

                for q in range(QG):
                    nc.tensor.matmul(
                        psum,
                        oh[:, q * 128:(q + 1) * 128],
                        ul[:, q, :],
                        start=(i == 0 and q == 0),
                        stop=(i == NT - 1 and q == QG - 1),
                    )

            out_sb = outp.tile([P, NCOL], f32)
            nc.vector.tensor_copy(out=out_sb, in_=psum)
            nc.sync.dma_start(out=out, in_=out_sb)

    nc.compile()
    return nc


def _get_program():
    global _PROG
    if _PROG is None:
        _PROG = _build_program()
    return _PROG


def _dist_tables():
    j = np.arange(C, dtype=np.float64)
    c = np.arange(C, dtype=np.float64)
    # match reference f32 rounding of |j-t|**6, then combine in f64
    dist = (np.abs(j[None, :] - c[:, None]) ** ALPHA).astype(np.float32)
    dist = dist.astype(np.float64)
    return dist, dist.sum(axis=1)


def _run(inputs, trace=False):
    from concourse.bass_utils import run_bass_kernel_spmd

    x_full = np.ascontiguousarray(np.asarray(inputs["outputs"], dtype=np.float32))
    t_full = np.asarray(inputs["targets"])
    assert x_full.shape == (B, C), x_full.shape
    t_f32 = np.ascontiguousarray(t_full.astype(np.float32).reshape(B))
    iota_host = np.ascontiguousarray(
        np.tile(np.arange(C, dtype=np.float32), (P, 1)))

    xs = x_full.reshape(N_CORES, B_LOCAL, C)
    ts = t_f32.reshape(N_CORES, B_LOCAL)
    in_maps = [
        {"x": np.ascontiguousarray(xs[ci]),
         "t": np.ascontiguousarray(ts[ci]),
         "iota": iota_host}
        for ci in range(N_CORES)
    ]

    nc = _get_program()
    res = run_bass_kernel_spmd(nc, in_maps, core_ids=list(range(N_CORES)),
                               trace=trace)

    ptot = np.zeros((P, NCOL), dtype=np.float64)
    for m in res.results:
        ptot += m["out"].astype(np.float64)

    dist, r = _dist_tables()
    msum = np.zeros((C, C), dtype=np.float64)
    nsum = np.zeros(C, dtype=np.float64)
    for gh in range(4):
        msum += ptot[gh * 32:(gh + 1) * 32, gh * 32:(gh + 1) * 32]
        nsum += ptot[gh * 32:(gh + 1) * 32, 128 + gh]
    loss = (np.dot(nsum, r) - np.sum(msum * dist)) / B
    return np.float32(loss), res


def kernel(**inputs) -> np.ndarray:
    loss, _ = _run(inputs, trace=False)
    return np.asarray(loss, dtype=np.float32)


# revision 9
# speedup vs baseline: 1.0490x; 1.0490x over previous
"""CDWCE loss kernel for Trainium2 (8 NeuronCores, data-parallel over batch).

Math: loss = mean_b sum_j -log(1 - softmax(x)_bj + eps) * |j - t_b|^6
With u_bj = s_b - e_bj (s = row sum of exp), the per-element term is
v_bj = ln(s_b) - ln(u_bj)  (= -ln(1 - p_bj); the 1e-7 eps shifts the
reference value by <= ~1e-7 relative, far below tolerance).

dist |j-t|^6 only depends on the target class, so the dist-weighted sum is
applied AFTER aggregating per-class sums of ln(u) rows:
  loss*B = sum_c [ R_c * sum_{b:t=c} ln s_b ] - sum_{c,j} DIST[c,j] * sum_{b:t=c} ln u_bj
The per-class sums are computed on-chip with one-hot matmuls accumulating in
PSUM; the tiny 32x32 DIST contraction happens on the host in float64.

Engine split per tile: ACT exp+ln, DVE grouped reduce + one-hot compare,
GPSIMD broadcast-subtract (runs concurrently with DVE), PE scatter matmuls.
"""

import numpy as np

B, C = 1048576, 32
N_CORES = 8
B_LOCAL = B // N_CORES          # 131072 rows per core
P = 128                         # SBUF partitions
G = 128                         # rows per partition per tile
NT = B_LOCAL // (P * G)         # 8 tiles per core
F = G * C                       # 4096 f32 per partition per tile
WG = G // 8                     # 16 matmul row-groups per tile (8 rows each)
NCOL = 264                      # 8*32 ln(u) cols + 8 ln(s) cols per row-group
ALPHA = 6
W_POOL = 16                     # how many of the WG u-subtract groups go to GPSIMD

_PROG = None


def _patch_act_tables():
    """Force exp+ln onto the shared 'natural_log_exp_and_others' table set so
    interleaved exp/ln activations don't reload ACT tables every tile.
    Emptying the competing sets (instead of removing them) keeps
    act_func_set_id indices aligned with act_info.json."""
    import concourse.hw_specs as hw_specs
    from concourse import mybir

    if getattr(hw_specs.get_activation_tables, "_cdwce_patched", False):
        return
    AF = mybir.ActivationFunctionType
    orig = hw_specs.get_activation_tables

    def patched(arch):
        t = orig(arch)
        combined = "natural_log_exp_and_others"
        if combined in t and AF.Exp in t[combined] and AF.Ln in t[combined]:
            for k in list(t):
                if k != combined and (AF.Exp in t[k] or AF.Ln in t[k]):
                    t[k] = set()
        return t

    patched._cdwce_patched = True
    hw_specs.get_activation_tables = patched
    import concourse.bacc as bacc_mod

    if hasattr(bacc_mod, "get_activation_tables"):
        bacc_mod.get_activation_tables = patched


def _build_program():
    import concourse.bass as bass
    import concourse.bacc as bacc
    import concourse.tile as tile
    from concourse import mybir

    _patch_act_tables()
    AF = mybir.ActivationFunctionType
    Alu = mybir.AluOpType
    f32 = mybir.dt.float32
    f32r = mybir.dt.float32r
    bf16 = mybir.dt.bfloat16

    nc = bacc.Bacc("TRN2", target_bir_lowering=False, debug=False,
                   enable_asserts=True)
    x = nc.dram_tensor("x", [B_LOCAL, C], f32, kind="ExternalInput").ap()
    t = nc.dram_tensor("t", [B_LOCAL], f32, kind="ExternalInput").ap()
    iota = nc.dram_tensor("iota", [P, C], f32, kind="ExternalInput").ap()
    out = nc.dram_tensor("out", [P, 2, NCOL], f32, kind="ExternalOutput").ap()

    # row index = n*(P*G) + p*G + g  ->  tile n, partition p, row-slot g
    xr = x.rearrange("(n p g) c -> n p (g c)", p=P, g=G)   # [NT, 128, F]
    tr = t.rearrange("(n p g) -> p n g", p=P, g=G)         # [128, NT, G]

    with tile.TileContext(nc) as tc:
        with (
            tc.tile_pool(name="consts", bufs=1) as consts,
            tc.tile_pool(name="xin", bufs=3) as xin,
            tc.tile_pool(name="work", bufs=2) as work,
            tc.tile_pool(name="smalls", bufs=2) as smalls,
            tc.tile_pool(name="psump", bufs=1, space="PSUM") as psump,
            tc.tile_pool(name="outp", bufs=1) as outp,
        ):
            iota_sb = consts.tile([P, C], f32)
            nc.sync.dma_start(out=iota_sb, in_=iota)
            t_sb = consts.tile([P, NT, G], f32)
            nc.sync.dma_start(out=t_sb, in_=tr)

            psum_a = psump.tile([P, NCOL], mybir.dt.float32)
            psum_b = psump.tile([P, NCOL], mybir.dt.float32)

            for i in range(NT):
                xt = xin.tile([P, F], f32)
                nc.sync.dma_start(out=xt, in_=xr[i])

                et = work.tile([P, F], f32)
                nc.scalar.activation(out=et, in_=xt, func=AF.Exp)

                st = smalls.tile([P, G], f32)
                nc.vector.reduce_sum(
                    out=st,
                    in_=et.rearrange("p (g c) -> p g c", c=C),
                    axis=mybir.AxisListType.X,
                )

                # u = s - e in f32; ln(u) is written as bf16 into ul
                # ul[:, w, 0:256] = ln(u) for the 8 row-slots of group w
                # ul[:, w, 256:264] = ln(s) for those row-slots
                ut = work.tile([P, F], f32)
                ul = work.tile([P, WG, NCOL], bf16)
                s8 = st.rearrange("p (w h) -> p w h", h=8)          # [P,WG,8]
                s8b = s8.unsqueeze(3).to_broadcast([P, WG, 8, C])
                e8 = et.rearrange("p (w h c) -> p w h c", h=8, c=C)
                u8 = ut.rearrange("p (w h c) -> p w h c", h=8, c=C)
                if W_POOL > 0:
                    nc.gpsimd.tensor_tensor(
                        out=u8[:, 0:W_POOL], in0=s8b[:, 0:W_POOL],
                        in1=e8[:, 0:W_POOL], op=Alu.subtract)
                if W_POOL < WG:
                    nc.vector.tensor_tensor(
                        out=u8[:, W_POOL:], in0=s8b[:, W_POOL:],
                        in1=e8[:, W_POOL:], op=Alu.subtract)

                nc.scalar.activation(
                    out=ul[:, :, 0:256].rearrange("p w (h c) -> p w h c", c=C),
                    in_=u8, func=AF.Ln)
                nc.scalar.activation(out=ul[:, :, 256:264], in_=s8, func=AF.Ln)

                # one-hot: oh[p, g*32+c] = (t[p,g] == c)
                oh = work.tile([P, F], bf16)
                ib = iota_sb.unsqueeze(1).to_broadcast([P, G, C])
                tb = t_sb[:, i, :].unsqueeze(2).to_broadcast([P, G, C])
                nc.vector.tensor_tensor(
                    out=oh.rearrange("p (g c) -> p g c", c=C),
                    in0=ib, in1=tb, op=Alu.is_equal)

                for w in range(WG):
                    rhs = ul[:, w, :]
                    nc.tensor.matmul(
                        psum_a,
                        oh[:, (8 * w) * C:(8 * w) * C + 128],
                        rhs,
                        start=(i == 0 and w == 0),
                        stop=(i == NT - 1 and w == WG - 1),
                    )
                    nc.tensor.matmul(
                        psum_b,
                        oh[:, (8 * w + 4) * C:(8 * w + 4) * C + 128],
                        rhs,
                        start=(i == 0 and w == 0),
                        stop=(i == NT - 1 and w == WG - 1),
                    )

            out_sb = outp.tile([P, 2, NCOL], f32)
            nc.vector.tensor_copy(out=out_sb[:, 0, :], in_=psum_a)
            nc.vector.tensor_copy(out=out_sb[:, 1, :], in_=psum_b)
            nc.sync.dma_start(out=out, in_=out_sb)

    nc.compile()
    return nc


def _get_program():
    global _PROG
    if _PROG is None:
        _PROG = _build_program()
    return _PROG


def _dist_tables():
    j = np.arange(C, dtype=np.float64)
    c = np.arange(C, dtype=np.float64)
    # match reference f32 rounding of |j-t|**6, then combine in f64
    dist = (np.abs(j[None, :] - c[:, None]) ** ALPHA).astype(np.float32)
    dist = dist.astype(np.float64)
    return dist, dist.sum(axis=1)


def _run(inputs, trace=False):
    from concourse.bass_utils import run_bass_kernel_spmd

    x_full = np.ascontiguousarray(np.asarray(inputs["outputs"], dtype=np.float32))
    t_full = np.asarray(inputs["targets"])
    assert x_full.shape == (B, C), x_full.shape
    t_host = np.ascontiguousarray(t_full.reshape(B).astype(np.float32))
    iota_host = np.ascontiguousarray(np.tile(np.arange(C, dtype=np.float32), (P, 1)))

    xs = x_full.reshape(N_CORES, B_LOCAL, C)
    ts = t_host.reshape(N_CORES, B_LOCAL)
    in_maps = [
        {"x": np.ascontiguousarray(xs[ci]),
         "t": np.ascontiguousarray(ts[ci]),
         "iota": iota_host}
        for ci in range(N_CORES)
    ]

    nc = _get_program()
    res = run_bass_kernel_spmd(nc, in_maps, core_ids=list(range(N_CORES)),
                               trace=trace)

    # out[:, 0, :] accumulates row-slots 0-3 of each 8-group (psum A),
    # out[:, 1, :] row-slots 4-7 (psum B).
    pa = np.zeros((P, NCOL), dtype=np.float64)
    pb = np.zeros((P, NCOL), dtype=np.float64)
    for m in res.results:
        o = m["out"].astype(np.float64)
        pa += o[:, 0, :]
        pb += o[:, 1, :]

    dist, r = _dist_tables()
    msum = np.zeros((C, C), dtype=np.float64)
    nsum = np.zeros(C, dtype=np.float64)
    for gh in range(4):
        # psum A: lhsT slot gh covers row-slot gh; useful rhs block at
        # cols 32*gh..32*gh+31, lns at col 256+gh
        msum += pa[gh * 32:(gh + 1) * 32, gh * 32:(gh + 1) * 32]
        nsum += pa[gh * 32:(gh + 1) * 32, 256 + gh]
        # psum B: lhsT slot gh covers row-slot 4+gh; useful rhs block at
        # cols 32*(4+gh).., lns at col 256+4+gh
        msum += pb[gh * 32:(gh + 1) * 32, (4 + gh) * 32:(5 + gh) * 32]
        nsum += pb[gh * 32:(gh + 1) * 32, 260 + gh]
    loss = (np.dot(nsum, r) - np.sum(msum * dist)) / B
    return np.float32(loss), res


def kernel(**inputs) -> np.ndarray:
    loss, _ = _run(inputs, trace=False)
    return np.asarray(loss, dtype=np.float32)
